# revision 9
# baseline (speedup 1.0000x reference)
"""Masked dot-product attention on 8 Trainium2 NeuronCores (Bass/Tile).

Problem: B=8, Nq=2048, Nk=2048, D=64 fp32; per-batch valid_lens L_b masks
keys k >= L_b before softmax.

Strategy (v3): fp16 SPMD with uniform per-core segment structure.
 - Work unit ("pair-task") = (batch b, query-half qh of 1024 queries,
   pair of 128-key chunks). Only chunks below ceil(L_b/128) are computed.
 - Each core runs S pair-tasks grouped into NSEG segments; a segment is a
   contiguous pair range of one (b, qh) block, accumulating its AV partial
   in PSUM across the segment (ones-column of V' gives the softmax
   denominator in row 64). Host adds partials of split blocks + divides.
 - Scores: PE pair-trick, 2 concurrent 64-contraction fp16 matmuls
   (tile_position (0,0)/(64,0)) -> PSUM [128 keys, 512 q].
 - exp: alternated between ACT (exact, scale=0.125, fp16 out) and DVE
   (Schraudolph: int16(round(s*184.665 + B)) bit-cast to fp16).
 - AV: stationary V' [128 keys, 65] fp16 (64 dims + ones col), moving e2
   fp16 [128, 512], accumulated in PSUM U [65, 512] over the segment.
 - v3 wire diet: V chunks ship 65 cols (not 128 zero-padded); Q for
   segments >= 1 ships once at 64 rows and is duplicated to partitions
   64:128 with an SBUF->SBUF DMA (PE pair-trick needs both row halves).
   Per-core input ~0.96MB (was 1.57MB).
 - v3 tail diet: final-segment PSUM->SBUF copy split across ACT+DVE,
   final output DMA split across sync+scalar HWDGE queues, drain waits
   batched on one NoOp chain (bigw), LAG 3.
 - Warmup: few WIDE (512-col) matmuls spin the HAM activity window at
   high intensity during the DMA wait.
"""
import math
import os
import sys

for _p in ("/opt/trn_rl_repo", "/root/.axon_site/_ro/trn_rl_repo"):
    if os.path.isdir(_p):
        if _p not in sys.path:
            sys.path.insert(0, _p)
        break

import numpy as np

import bass_rust
import concourse.bass as bass
import concourse.tile as tile
from concourse import mybir
from concourse.bass_utils import run_bass_kernel_spmd

F32 = mybir.dt.float32
F16 = mybir.dt.float16
I16 = mybir.dt.int16

B, NQ, NK, D = 8, 2048, 2048, 64
QH = 1024                   # queries per block (query-half)
QG = 512                    # matmul moving width (PSUM bank = 512 f32)
VC = 65                     # V chunk cols on the wire (64 dims + ones)
NCORE = 8
LOG2E = 1.4426950408889634
A_DVE = 128.0 * LOG2E       # exp(s/8) = 2^(s*A/1024)
C_DVE = -59.55              # centering constant (weighted mean rel-err ~ 0)
B_DVE = 15.0 * 1024.0 + C_DVE

def _split_waits(nc, maxw=1):
    """Walrus in this container rejects >1 sync wait per instruction;
    hoist excess waits onto NoOps inserted just before.

    The FINAL drain (tile exit) is special-cased: its excess waits are
    distributed round-robin onto end-of-program NoOps across ALL engines
    so they resolve in parallel (~5x shorter serial tail). The walrus
    postamble's entry barrier joins all engines, so the happens-before
    relation to end-of-kernel is preserved."""
    cnt = 0
    last_drain = None
    for f in nc.m.functions:
        for bb in f.blocks:
            for ins in bb.instructions:
                if (type(ins).__name__ == "InstDrain"
                        and str(ins.engine).endswith("SP")):
                    last_drain = (bb, ins)
    if last_drain is not None:
        bb, drain = last_drain
        si = drain.sync_info
        waits = list(si.on_wait) if si is not None and si.on_wait else []
        if len(waits) > 1:
            engines = sorted({str(i2.engine) for i2 in bb.instructions
                              if getattr(i2, "engine", None) is not None})
            keep = waits[-1:]
            for j, w in enumerate(waits[:-1]):
                cnt += 1
                nop = mybir.InstNoOp(name=f"I-wd{cnt}", ins=[], outs=[])
                eng_ins = [i2 for i2 in bb.instructions
                           if str(i2.engine) == engines[j % len(engines)]]
                nop.engine = eng_ins[-1].engine
                nop.sync_info = bass_rust.SyncInfo(on_wait=[w], on_update=[])
                bb.instructions.append(nop)
            drain.sync_info = bass_rust.SyncInfo(
                on_wait=keep, on_update=list(si.on_update or []))
    for f in nc.m.functions:
        for bb in f.blocks:
            insts = bb.instructions
            i = 0
            while i < len(insts):
                ins = insts[i]
                si = ins.sync_info
                waits = list(si.on_wait) if si is not None and si.on_wait else []
                if len(waits) > maxw:
                    keep = waits[len(waits) - maxw:]
                    excess = waits[: len(waits) - maxw]
                    for j in range(0, len(excess), maxw):
                        cnt += 1
                        nop = mybir.InstNoOp(name=f"I-ws{cnt}", ins=[], outs=[])
                        nop.engine = ins.engine
                        nop.sync_info = bass_rust.SyncInfo(
                            on_wait=excess[j : j + maxw], on_update=[]
                        )
                        insts.insert(i, nop)
                        i += 1
                    ins.sync_info = bass_rust.SyncInfo(
                        on_wait=keep, on_update=list(si.on_update or [])
                    )
                i += 1
    return cnt


def _dedup_waits(nc):
    """Drop waits already guaranteed by an earlier instruction on the same
    in-order engine (all waits are sem-ge-imm, sems are monotonic within an
    execution). Never strips InstLdweights — the PE queue may hoist those
    ahead of in-flight matmuls, so their own waits must stay."""
    dropped = 0
    for f in nc.m.functions:
        for bb in f.blocks:
            seen = {}
            for ins in bb.instructions:
                si = ins.sync_info
                if si is None or not si.on_wait:
                    continue
                eng = str(ins.engine)
                is_lw = type(ins).__name__ == "InstLdweights"
                keep = []
                changed = False
                for w in si.on_wait:
                    ok_kind = (str(w.sync_type) == "semaphore"
                               and str(w.wait_mode) == "sem-ge-imm"
                               and w.wait_value is not None)
                    key = (eng, w.ant_name)
                    if (ok_kind and not is_lw
                            and w.wait_value <= seen.get(key, -1)):
                        dropped += 1
                        changed = True
                        continue
                    keep.append(w)
                    if ok_kind:
                        seen[key] = max(seen.get(key, -1), w.wait_value)
                if changed:
                    ins.sync_info = bass_rust.SyncInfo(
                        on_wait=keep, on_update=list(si.on_update or []))
    return dropped


class _SlimTileContext(tile.TileContext):
    """Skip the exit sem-clears + double barrier (sems re-init at entry)."""

    def _drain_and_barrier(self, tick_clock, wait_clock):
        from concourse.vector_clock import ScopedClock
        drain_inst = self.nc.sync.drain()
        wait_clock.add_sem_waits(
            drain_inst.ins, ScopedClock({None: tick_clock.global_clock})
        )
        popped = self.nc._tile_sem_poison_stack.pop()
        assert popped is self._sem_poison


_BUILT = {}


def _layout(pattern):
    """Need-ordered input blob layout + DMA unit ranges.

    Returns (offs, units, X): offs[(kind, idx)] = start col of 'q' seg /
    'k' pair / 'v' chunk; units = [(lo, hi, eng, rows)] DMA ranges in
    issue order, eng 0 = sync, 1 = gpsimd; rows = partition rows shipped
    (128, or 64 for deduplicated q segments)."""
    offs = {}
    units = []
    pos = 0
    n0p = min(2, pattern[0])

    def put(kind, idx, n):
        nonlocal pos
        offs[(kind, idx)] = pos
        pos += n

    def pair_cols(p):
        put("k", p, 128)
        put("v", 2 * p, VC)
        put("v", 2 * p + 1, VC)

    # minimal first-iteration set in parallel on both queues:
    # sync: q0 first half (full 128 rows); gpsimd: pair 0, then q0 second
    # half, then pair 1
    put("q", 0, QH)
    units.append((0, QG, 0, 128))
    start = pos
    for p in range(min(1, n0p)):
        pair_cols(p)
    units.append((start, pos, 1, 128))
    units.append((QG, QH, 0, 128))
    start = pos
    for p in range(1, n0p):
        pair_cols(p)
    if pos > start:
        units.append((start, pos, 1, 128))
    # segment 0 leftovers in groups of 3 pairs
    ps = [p for p in range(pattern[0]) if p >= n0p]
    for g in range(0, len(ps), 3):
        start = pos
        for p in ps[g : g + 3]:
            pair_cols(p)
        units.append((start, pos, 1, 128))
    # q segments 1+ ship 64 rows once (dup to rows 64:128 on-chip)
    qstart = pos
    for sgi in range(1, len(pattern)):
        put("q", sgi, QH)
    if pos > qstart:
        units.append((qstart, pos, 0, 64))
    # all remaining pairs as one big gpsimd unit
    start = pos
    plo = pattern[0]
    for sgi, sz in enumerate(pattern):
        if sgi == 0:
            continue
        for p in range(plo, plo + sz):
            pair_cols(p)
        plo += sz
    if pos > start:
        units.append((start, pos, 1, 128))
    # 63-col pad: AV stationaries read 128 cols from 65-col V slots; the
    # last slot's over-read must stay in-bounds AND be DMA-written (race
    # detector). Extend the last full-row unit to cover it.
    pos += 63
    for ui in range(len(units) - 1, -1, -1):
        lo, hi, eng, rows = units[ui]
        if rows == 128:
            if ui == len(units) - 1:
                units[ui] = (lo, pos, eng, rows)
            else:
                units.append((hi, pos, eng, 128))
            break
    return offs, units, pos


def _build(pattern, slim=True, exp_mode="mix", lag=3, dup_dma=True,
           splitfin=True, scalar_dma=True, wide_warm=True):
    """pattern: tuple of segment sizes in pairs, e.g. (6, 2, 1)."""
    NSEG = len(pattern)
    S = sum(pattern)                      # pairs per core
    nc = bass.Bass(trn_type="TRN2")
    offs, units, X = _layout(pattern)
    bx = nc.dram_tensor("bx", [128, X], F16, kind="ExternalInput")
    po = nc.dram_tensor("po", [NSEG, VC, 2, QG], F16, kind="ExternalOutput")

    # half-iter -> (seg, pair-in-core, h, first_pair_of_seg, last_pair_of_seg)
    iters = []
    p0 = 0
    for sgi, sz in enumerate(pattern):
        for p in range(sz):
            for h in range(2):
                iters.append((sgi, p0 + p, h, p == 0, p == sz - 1))
        p0 += sz

    ctx_cls = _SlimTileContext if slim else tile.TileContext
    with ctx_cls(nc) as tc:
        with (
            tc.tile_pool(name="ipool", bufs=1) as ipool,
            tc.tile_pool(name="epool", bufs=6) as epool,
            tc.tile_pool(name="usb", bufs=2) as usbp,
            tc.tile_pool(name="s2pool", bufs=3, space="PSUM") as s2pool,
            tc.tile_pool(name="upool", bufs=2, space="PSUM") as upool,
        ):
            # ACT warmup: force the Exp table load during the DMA wait.
            wsb = ipool.tile([128, 640], F16, tag="warm")
            nc.gpsimd.memset(wsb[:], 0.0)
            wact = ipool.tile([128, 128], F16, tag="wact")
            nc.scalar.activation(wact[:], wsb[:, 0:128],
                                 mybir.ActivationFunctionType.Exp, scale=0.125)
            # PE warmup: few WIDE matmuls keep the HAM activity window at
            # high intensity during the input-DMA wait.
            wps = s2pool.tile([128, 2, QG], F32, tag="s2")
            if wide_warm:
                for w in range(5):
                    nc.tensor.matmul(wps[:, w % 2, :], wsb[:, 0:128],
                                     wsb[:, 128:640], start=True, stop=True)
            else:
                for w in range(21):
                    nc.tensor.matmul(wps[:, w % 2, 0:128], wsb[:, 0:128],
                                     wsb[:, 128:256], start=True, stop=True)

            tx = ipool.tile([128, X], F16, tag="tx")
            for lo, hi, eng, rows in units:
                e = nc.sync if eng == 0 else nc.gpsimd
                if not dup_dma:
                    rows = 128
                e.dma_start(tx[0:rows, lo:hi], bx[0:rows, lo:hi])
            # duplicate q segments 1+ to partition rows 64:128 (fabric,
            # not HBM) for the PE pair-trick's second row-half.
            if dup_dma:
                for sgi in range(1, NSEG):
                    off = offs[("q", sgi)]
                    nc.sync.dma_start(tx[64:128, off : off + QH],
                                      tx[0:64, off : off + QH])

            def qap(seg):
                off = offs[("q", seg)]
                return tx[:, off : off + QH]

            def kap(p):
                off = offs[("k", p)]
                return tx[:, off : off + 128]

            def vap(ch):
                # 65 real cols; over-read to 128 so the stationary shape
                # matches the baseline (walrus rejects 65-col weights).
                # Out partitions 65:128 of U are garbage and never read.
                off = offs[("v", ch)]
                return tx[:, off : off + 128]

            # engine picker for exp/copy ops: static least-loaded
            eng_load = [0.0, 0.0]         # ACT, DVE (measured ns per 1024-col op)
            ECOST = [1336.0, 1469.0]

            def pick_engine():
                if exp_mode == "act":
                    return 0
                if exp_mode == "dve":
                    return 1
                e = 0 if eng_load[0] + ECOST[0] <= eng_load[1] + ECOST[1] else 1
                eng_load[e] += ECOST[e]
                return e

            LAG = lag
            e2s = {}
            u_half = [None, None]
            u_outs = {}
            for i in range(len(iters) + LAG):
                if i < len(iters):
                    sgi, p, h, first, last = iters[i]
                    s2 = s2pool.tile([128, 2, QG], F32, name=f"s2_{i}",
                                     tag="s2")
                    ktile = kap(p)
                    qm = qap(sgi)[:, h * QG : (h + 1) * QG]
                    nc.tensor.matmul(s2[:, 0, :], ktile[0:64, :],
                                     qm[0:64, :], start=True, stop=True,
                                     tile_position=(0, 0))
                    nc.tensor.matmul(s2[:, 1, :], ktile[64:128, :],
                                     qm[64:128, :], start=True, stop=True,
                                     tile_position=(64, 0))
                    e2 = epool.tile([128, 2, QG], F16, name=f"e2_{i}",
                                    tag="e2")
                    if i >= len(iters) - 2 or i < 2:
                        # pipeline fill and drain iterations: split exp
                        # across both engines — shortens the critical chain
                        nc.scalar.activation(
                            e2[:, 0, :], s2[:, 0, :],
                            mybir.ActivationFunctionType.Exp, scale=0.125)
                        nc.vector.tensor_scalar(
                            e2[:, 1, :].bitcast(I16), s2[:, 1, :], A_DVE,
                            B_DVE, mybir.AluOpType.mult, mybir.AluOpType.add)
                    elif pick_engine() == 0:
                        nc.scalar.activation(
                            e2[:], s2[:],
                            mybir.ActivationFunctionType.Exp, scale=0.125)
                    else:
                        nc.vector.tensor_scalar(
                            e2[:].bitcast(I16), s2[:], A_DVE, B_DVE,
                            mybir.AluOpType.mult, mybir.AluOpType.add)
                    e2s[i] = e2
                if i >= LAG:
                    j = i - LAG
                    sgi, p, h, first, last = iters[j]
                    e2 = e2s.pop(j)
                    if first:
                        u_half[h] = upool.tile([128, QG], F32,
                                               name=f"u_{j}", tag="u")
                    ut = u_half[h]
                    # alternate chunk order by half so consecutive AVs share
                    # a stationary (B,h0 -> B,h1) when adjacent
                    chunks = ((0, 1) if h == 0 else (1, 0))
                    for ci, cc in enumerate(chunks):
                        nc.tensor.matmul(ut[:], vap(2 * p + cc),
                                         e2[:, cc, :],
                                         start=(first and ci == 0),
                                         stop=(last and ci == 1))
                    if last:
                        # evacuate this half as soon as its last AV is done
                        if h == 0:
                            u_out = usbp.tile([VC, 2, QG], F16,
                                              name=f"uo_{sgi}", tag="uo")
                            u_outs[sgi] = u_out
                        uo = u_outs[sgi]
                        if sgi == NSEG - 1 and splitfin:
                            # final segment: split the copy across both
                            # engines and the DMA across both HWDGE queues
                            # to shorten the tail
                            nc.scalar.copy(uo[:, h, 0:256], ut[0:VC, 0:256])
                            nc.vector.tensor_copy(uo[:, h, 256:512],
                                                  ut[0:VC, 256:512])
                            nc.sync.dma_start(po[sgi][:, h, 0:256],
                                              uo[:, h, 0:256])
                            e2nd = nc.scalar if scalar_dma else nc.sync
                            e2nd.dma_start(po[sgi][:, h, 256:512],
                                           uo[:, h, 256:512])
                        elif sgi == NSEG - 1:
                            if pick_engine() == 0:
                                nc.scalar.copy(uo[:, h, :], ut[0:VC, :])
                            else:
                                nc.vector.tensor_copy(uo[:, h, :], ut[0:VC, :])
                            nc.sync.dma_start(po[sgi][:, h, :], uo[:, h, :])
                        else:
                            if pick_engine() == 0:
                                nc.scalar.copy(uo[:, h, :], ut[0:VC, :])
                            else:
                                nc.vector.tensor_copy(uo[:, h, :], ut[0:VC, :])
                            if h == 1:
                                nc.sync.dma_start(po[sgi][:], uo[:])

    _dedup_waits(nc)
    _split_waits(nc)
    return nc


def _plan(valid_lens):
    """Pack (b, qh) blocks (weight = ceil(nchunks/2) pairs) into NCORE
    cores x segment slots. Returns (pattern, assign) where assign[c] is a
    list of (slot, b, qh, pair_lo, npairs) with slot < len(pattern)."""
    blocks = []
    for b in range(B):
        nch = max(1, math.ceil(int(valid_lens[b]) / 128))
        npair = (nch + 1) // 2
        for qh in range(2):
            blocks.append([npair, b, qh])
    total = sum(bl[0] for bl in blocks)
    S = max(1, math.ceil(total / NCORE))

    def try_pattern(pat):
        slots = []
        for c in range(NCORE):
            for si, cap in enumerate(pat):
                slots.append([cap, c, si])
        rem = sorted(([bl[0], bl[1], bl[2], 0] for bl in blocks),
                     reverse=True)
        assign = [[] for _ in range(NCORE)]
        waste = 0
        slots.sort(reverse=True)
        used = [False] * len(slots)
        for _ in range(1000):
            rem = [r for r in rem if r[0] > 0]
            if not rem:
                break
            rem.sort(reverse=True)
            r = rem[0]
            best = None
            for k, sl in enumerate(slots):
                if used[k]:
                    continue
                if best is None or sl[0] > slots[best][0]:
                    best = k
            if best is None:
                return None, None
            cap, c, si = slots[best]
            used[best] = True
            take = min(cap, r[0])
            assign[c].append((si, r[1], r[2], r[3], take))
            waste += cap - take
            r[3] += take
            r[0] -= take
        else:
            return None, None
        return waste, assign

    best = None
    cands = []
    for a in range(S, 0, -1):
        for bb in range(S - a, -1, -1):
            c = S - a - bb
            if bb and c > bb:
                continue
            if not bb and c:
                continue
            pat = tuple(x for x in (a, bb, c) if x > 0)
            cands.append(pat)
    for pat in cands:
        waste, assign = try_pattern(pat)
        if waste is None:
            continue
        key = (waste, len(pat))
        if best is None or key < best[0]:
            best = (key, pat, assign)
    assert best is not None
    return best[1], best[2]


def _host_prep(queries, keys, values, valid_lens, pattern, assign):
    queries = np.asarray(queries, dtype=np.float32)
    keys = np.asarray(keys, dtype=np.float32)
    values = np.asarray(values, dtype=np.float32)
    NSEG = len(pattern)
    S = sum(pattern)
    slot_lo = np.cumsum([0] + list(pattern))

    qts = queries.transpose(0, 2, 1).astype(np.float16)   # [B, 64, 2048]
    kts = keys.transpose(0, 2, 1).astype(np.float16)      # [B, 64, 2048]
    vps = np.zeros((B, NK + 256, VC), dtype=np.float16)   # [keys, 65]
    for b in range(B):
        L = int(valid_lens[b])
        vps[b, :L, :D] = values[b][:L].astype(np.float16)
        vps[b, :L, D] = 1.0

    offs, units, X = _layout(tuple(pattern))
    in_maps = []
    for c in range(NCORE):
        qblob = np.zeros((128, NSEG, QH), dtype=np.float16)
        kblob = np.zeros((128, S, 128), dtype=np.float16)
        vblob = np.zeros((128, 2 * S, VC), dtype=np.float16)
        for (si, b, qh, plo, np_) in assign[c]:
            qsl = qts[b][:, qh * QH : (qh + 1) * QH]
            qblob[0:64, si, :] = qsl
            # rows 64:128 are only DMA'd for seg 0 (dup_dma mode); filling
            # them always keeps the blob valid for either build flavor
            qblob[64:128, si, :] = qsl
            for pp in range(np_):
                gp = slot_lo[si] + pp           # global pair slot on core
                for half in range(2):
                    ch = (plo + pp) * 2 + half
                    c0, c1 = ch * 128, (ch + 1) * 128
                    c1v = min(c1, int(valid_lens[b]))
                    if c0 < NK:
                        kblob[64 * half : 64 * half + 64, gp, :] = (
                            kts[b][:, c0:c1])
                        # zero masked key columns (>= L)
                        if c1v < c1:
                            kblob[64 * half : 64 * half + 64, gp,
                                  max(0, c1v - c0):] = 0
                        vblob[:, 2 * gp + half, :] = vps[b][c0:c1, :]
        bxb = np.zeros((128, X), dtype=np.float16)
        for (kind, idx), off in offs.items():
            if kind == "q":
                bxb[:, off : off + QH] = qblob[:, idx, :]
            elif kind == "k":
                bxb[:, off : off + 128] = kblob[:, idx, :]
            else:
                bxb[:, off : off + VC] = vblob[:, idx, :]
        in_maps.append({"bx": bxb})
    return in_maps


def kernel(queries, keys, values, valid_lens):
    valid_lens = np.asarray(valid_lens)
    pattern, assign = _plan(valid_lens)
    key = tuple(pattern)
    if key not in _BUILT:
        _BUILT[key] = _build(pattern)
    in_maps = _host_prep(queries, keys, values, valid_lens, pattern, assign)
    res = run_bass_kernel_spmd(
        _BUILT[key],
        in_maps,
        core_ids=list(range(NCORE)),
        trace=bool(os.environ.get("KERNEL_TRACE")),
    )
    kernel.last_result = res

    # combine partials: ACC[b][qh] = sum over segments
    ACC = np.zeros((B, 2, VC, QH), dtype=np.float64)
    for c in range(NCORE):
        pc = np.asarray(res.results[c]["po"]).astype(np.float64)
        for (si, b, qh, plo, np_) in assign[c]:
            ACC[b, qh] += pc[si].reshape(VC, QH)
    out = np.empty((B, NQ, D), dtype=np.float32)
    for b in range(B):
        for qh in range(2):
            U = ACC[b, qh]
            out[b, qh * QH : (qh + 1) * QH, :] = (
                U[:D, :] / U[D : D + 1, :]).T.astype(np.float32)
    return out


kernel.last_result = None


# revision 10
# speedup vs baseline: 1.0202x; 1.0202x over previous
"""Masked dot-product attention on 8 Trainium2 NeuronCores (Bass/Tile).

Problem: B=8, Nq=2048, Nk=2048, D=64 fp32; per-batch valid_lens L_b masks
keys k >= L_b before softmax.

Strategy (v3): fp16 SPMD with uniform per-core segment structure.
 - Work unit ("pair-task") = (batch b, query-half qh of 1024 queries,
   pair of 128-key chunks). Only chunks below ceil(L_b/128) are computed.
 - Each core runs S pair-tasks grouped into NSEG segments; a segment is a
   contiguous pair range of one (b, qh) block, accumulating its AV partial
   in PSUM across the segment (ones-column of V' gives the softmax
   denominator in row 64). Host adds partials of split blocks + divides.
 - Scores: PE pair-trick, 2 concurrent 64-contraction fp16 matmuls
   (tile_position (0,0)/(64,0)) -> PSUM [128 keys, 512 q].
 - exp: alternated between ACT (exact, scale=0.125, fp16 out) and DVE
   (Schraudolph: int16(round(s*184.665 + B)) bit-cast to fp16).
 - AV: stationary V' [128 keys, 65] fp16 (64 dims + ones col), moving e2
   fp16 [128, 512], accumulated in PSUM U [65, 512] over the segment.
 - v3 wire diet: V chunks ship 65 cols (not 128 zero-padded); Q for
   segments >= 1 ships once at 64 rows and is duplicated to partitions
   64:128 with an SBUF->SBUF DMA (PE pair-trick needs both row halves).
   Per-core input ~0.96MB (was 1.57MB).
 - v3 tail diet: final-segment PSUM->SBUF copy split across ACT+DVE,
   final output DMA split across sync+scalar HWDGE queues, drain waits
   batched on one NoOp chain (bigw), LAG 3.
 - Warmup: few WIDE (512-col) matmuls spin the HAM activity window at
   high intensity during the DMA wait.
"""
import math
import os
import sys

for _p in ("/opt/trn_rl_repo", "/root/.axon_site/_ro/trn_rl_repo"):
    if os.path.isdir(_p):
        if _p not in sys.path:
            sys.path.insert(0, _p)
        break

import numpy as np

import bass_rust
import concourse.bass as bass
import concourse.tile as tile
from concourse import mybir
from concourse.bass_utils import run_bass_kernel_spmd

F32 = mybir.dt.float32
F16 = mybir.dt.float16
I16 = mybir.dt.int16

B, NQ, NK, D = 8, 2048, 2048, 64
QH = 1024                   # queries per block (query-half)
QG = 512                    # matmul moving width (PSUM bank = 512 f32)
VC = 65                     # V chunk cols on the wire (64 dims + ones)
NCORE = 8
LOG2E = 1.4426950408889634
A_DVE = 128.0 * LOG2E       # exp(s/8) = 2^(s*A/1024)
C_DVE = -59.55              # centering constant (weighted mean rel-err ~ 0)
B_DVE = 15.0 * 1024.0 + C_DVE

def _split_waits(nc, maxw=1):
    """Walrus in this container rejects >1 sync wait per instruction;
    hoist excess waits onto NoOps inserted just before.

    The FINAL drain (tile exit) is special-cased: its excess waits are
    distributed round-robin onto end-of-program NoOps across ALL engines
    so they resolve in parallel (~5x shorter serial tail). The walrus
    postamble's entry barrier joins all engines, so the happens-before
    relation to end-of-kernel is preserved."""
    cnt = 0
    last_drain = None
    for f in nc.m.functions:
        for bb in f.blocks:
            for ins in bb.instructions:
                if (type(ins).__name__ == "InstDrain"
                        and str(ins.engine).endswith("SP")):
                    last_drain = (bb, ins)
    if last_drain is not None:
        bb, drain = last_drain
        si = drain.sync_info
        waits = list(si.on_wait) if si is not None and si.on_wait else []
        if len(waits) > 1:
            engines = sorted({str(i2.engine) for i2 in bb.instructions
                              if getattr(i2, "engine", None) is not None})
            keep = waits[-1:]
            for j, w in enumerate(waits[:-1]):
                cnt += 1
                nop = mybir.InstNoOp(name=f"I-wd{cnt}", ins=[], outs=[])
                eng_ins = [i2 for i2 in bb.instructions
                           if str(i2.engine) == engines[j % len(engines)]]
                nop.engine = eng_ins[-1].engine
                nop.sync_info = bass_rust.SyncInfo(on_wait=[w], on_update=[])
                bb.instructions.append(nop)
            drain.sync_info = bass_rust.SyncInfo(
                on_wait=keep, on_update=list(si.on_update or []))
    for f in nc.m.functions:
        for bb in f.blocks:
            insts = bb.instructions
            i = 0
            while i < len(insts):
                ins = insts[i]
                si = ins.sync_info
                waits = list(si.on_wait) if si is not None and si.on_wait else []
                if len(waits) > maxw:
                    keep = waits[len(waits) - maxw:]
                    excess = waits[: len(waits) - maxw]
                    for j in range(0, len(excess), maxw):
                        cnt += 1
                        nop = mybir.InstNoOp(name=f"I-ws{cnt}", ins=[], outs=[])
                        nop.engine = ins.engine
                        nop.sync_info = bass_rust.SyncInfo(
                            on_wait=excess[j : j + maxw], on_update=[]
                        )
                        insts.insert(i, nop)
                        i += 1
                    ins.sync_info = bass_rust.SyncInfo(
                        on_wait=keep, on_update=list(si.on_update or [])
                    )
                i += 1
    return cnt


def _dedup_waits(nc):
    """Drop waits already guaranteed by an earlier instruction on the same
    in-order engine (all waits are sem-ge-imm, sems are monotonic within an
    execution). Never strips InstLdweights — the PE queue may hoist those
    ahead of in-flight matmuls, so their own waits must stay."""
    dropped = 0
    for f in nc.m.functions:
        for bb in f.blocks:
            seen = {}
            for ins in bb.instructions:
                si = ins.sync_info
                if si is None or not si.on_wait:
                    continue
                eng = str(ins.engine)
                is_lw = type(ins).__name__ == "InstLdweights"
                keep = []
                changed = False
                for w in si.on_wait:
                    ok_kind = (str(w.sync_type) == "semaphore"
                               and str(w.wait_mode) == "sem-ge-imm"
                               and w.wait_value is not None)
                    key = (eng, w.ant_name)
                    if (ok_kind and not is_lw
                            and w.wait_value <= seen.get(key, -1)):
                        dropped += 1
                        changed = True
                        continue
                    keep.append(w)
                    if ok_kind:
                        seen[key] = max(seen.get(key, -1), w.wait_value)
                if changed:
                    ins.sync_info = bass_rust.SyncInfo(
                        on_wait=keep, on_update=list(si.on_update or []))
    return dropped


class _SlimTileContext(tile.TileContext):
    """Skip the exit sem-clears + double barrier (sems re-init at entry)."""

    def _drain_and_barrier(self, tick_clock, wait_clock):
        from concourse.vector_clock import ScopedClock
        drain_inst = self.nc.sync.drain()
        wait_clock.add_sem_waits(
            drain_inst.ins, ScopedClock({None: tick_clock.global_clock})
        )
        popped = self.nc._tile_sem_poison_stack.pop()
        assert popped is self._sem_poison


_BUILT = {}


def _layout(pattern):
    """Need-ordered input blob layout + DMA unit ranges.

    Returns (offs, units, X): offs[(kind, idx)] = start col of 'q' seg /
    'k' pair / 'v' chunk; units = [(lo, hi, eng, rows)] DMA ranges in
    issue order, eng 0 = sync, 1 = gpsimd; rows = partition rows shipped
    (128, or 64 for deduplicated q segments)."""
    offs = {}
    units = []
    pos = 0
    n0p = min(2, pattern[0])

    def put(kind, idx, n):
        nonlocal pos
        offs[(kind, idx)] = pos
        pos += n

    def pair_cols(p):
        put("k", p, 128)
        put("v", 2 * p, VC)
        put("v", 2 * p + 1, VC)

    # minimal first-iteration set in parallel on both queues:
    # sync: q0 first half (full 128 rows); gpsimd: pair 0, then q0 second
    # half, then pair 1
    put("q", 0, QH)
    units.append((0, QG, 0, 128))
    start = pos
    for p in range(min(1, n0p)):
        pair_cols(p)
    units.append((start, pos, 1, 128))
    units.append((QG, QH, 0, 128))
    start = pos
    for p in range(1, n0p):
        pair_cols(p)
    if pos > start:
        units.append((start, pos, 1, 128))
    # segment 0 leftovers in groups of 3 pairs
    ps = [p for p in range(pattern[0]) if p >= n0p]
    for g in range(0, len(ps), 3):
        start = pos
        for p in ps[g : g + 3]:
            pair_cols(p)
        units.append((start, pos, 1, 128))
    # q segments 1+ ship 64 rows once (dup to rows 64:128 on-chip)
    qstart = pos
    for sgi in range(1, len(pattern)):
        put("q", sgi, QH)
    if pos > qstart:
        units.append((qstart, pos, 0, 64))
    # all remaining pairs as one big gpsimd unit
    start = pos
    plo = pattern[0]
    for sgi, sz in enumerate(pattern):
        if sgi == 0:
            continue
        for p in range(plo, plo + sz):
            pair_cols(p)
        plo += sz
    if pos > start:
        units.append((start, pos, 1, 128))
    # 63-col pad: AV stationaries read 128 cols from 65-col V slots; the
    # last slot's over-read must stay in-bounds AND be DMA-written (race
    # detector). Extend the last full-row unit to cover it.
    pos += 63
    for ui in range(len(units) - 1, -1, -1):
        lo, hi, eng, rows = units[ui]
        if rows == 128:
            if ui == len(units) - 1:
                units[ui] = (lo, pos, eng, rows)
            else:
                units.append((hi, pos, eng, 128))
            break
    return offs, units, pos


def _build(pattern, slim=True, exp_mode="mix", lag=3, dup_dma=True,
           splitfin=True, scalar_dma=True, wide_warm=True):
    """pattern: tuple of segment sizes in pairs, e.g. (6, 2, 1)."""
    NSEG = len(pattern)
    S = sum(pattern)                      # pairs per core
    nc = bass.Bass(trn_type="TRN2")
    offs, units, X = _layout(pattern)
    bx = nc.dram_tensor("bx", [128, X], F16, kind="ExternalInput")
    po = nc.dram_tensor("po", [NSEG, VC, 2, QG], F16, kind="ExternalOutput")

    # half-iter -> (seg, pair-in-core, h, first_pair_of_seg, last_pair_of_seg)
    iters = []
    p0 = 0
    for sgi, sz in enumerate(pattern):
        for p in range(sz):
            for h in range(2):
                iters.append((sgi, p0 + p, h, p == 0, p == sz - 1))
        p0 += sz

    ctx_cls = _SlimTileContext if slim else tile.TileContext
    with ctx_cls(nc) as tc:
        with (
            tc.tile_pool(name="ipool", bufs=1) as ipool,
            tc.tile_pool(name="epool", bufs=6) as epool,
            tc.tile_pool(name="usb", bufs=2) as usbp,
            tc.tile_pool(name="s2pool", bufs=3, space="PSUM") as s2pool,
            tc.tile_pool(name="upool", bufs=2, space="PSUM") as upool,
        ):
            # ACT warmup: force the Exp table load during the DMA wait.
            wsb = ipool.tile([128, 640], F16, tag="warm")
            nc.vector.memset(wsb[:], 0.0)
            wact = ipool.tile([128, 128], F16, tag="wact")
            nc.scalar.activation(wact[:], wsb[:, 0:128],
                                 mybir.ActivationFunctionType.Exp, scale=0.125)
            # PE warmup: few WIDE matmuls keep the HAM activity window at
            # high intensity during the input-DMA wait.
            wps = s2pool.tile([128, 2, QG], F32, tag="s2")
            if wide_warm:
                for w in range(5):
                    nc.tensor.matmul(wps[:, w % 2, :], wsb[:, 0:128],
                                     wsb[:, 128:640], start=True, stop=True)
            else:
                for w in range(21):
                    nc.tensor.matmul(wps[:, w % 2, 0:128], wsb[:, 0:128],
                                     wsb[:, 128:256], start=True, stop=True)

            tx = ipool.tile([128, X], F16, tag="tx")
            for lo, hi, eng, rows in units:
                e = nc.sync if eng == 0 else nc.gpsimd
                if not dup_dma:
                    rows = 128
                e.dma_start(tx[0:rows, lo:hi], bx[0:rows, lo:hi])
            # duplicate q segments 1+ to partition rows 64:128 (fabric,
            # not HBM) for the PE pair-trick's second row-half.
            if dup_dma:
                for sgi in range(1, NSEG):
                    off = offs[("q", sgi)]
                    nc.sync.dma_start(tx[64:128, off : off + QH],
                                      tx[0:64, off : off + QH])

            def qap(seg):
                off = offs[("q", seg)]
                return tx[:, off : off + QH]

            def kap(p):
                off = offs[("k", p)]
                return tx[:, off : off + 128]

            def vap(ch):
                # 65 real cols; over-read to 128 so the stationary shape
                # matches the baseline (walrus rejects 65-col weights).
                # Out partitions 65:128 of U are garbage and never read.
                off = offs[("v", ch)]
                return tx[:, off : off + 128]

            # engine picker for exp/copy ops: static least-loaded
            eng_load = [0.0, 0.0]         # ACT, DVE (measured ns per 1024-col op)
            ECOST = [1336.0, 1469.0]

            def pick_engine():
                if exp_mode == "act":
                    return 0
                if exp_mode == "dve":
                    return 1
                e = 0 if eng_load[0] + ECOST[0] <= eng_load[1] + ECOST[1] else 1
                eng_load[e] += ECOST[e]
                return e

            LAG = lag
            e2s = {}
            u_half = [None, None]
            u_outs = {}
            for i in range(len(iters) + LAG):
                if i < len(iters):
                    sgi, p, h, first, last = iters[i]
                    s2 = s2pool.tile([128, 2, QG], F32, name=f"s2_{i}",
                                     tag="s2")
                    ktile = kap(p)
                    qm = qap(sgi)[:, h * QG : (h + 1) * QG]
                    nc.tensor.matmul(s2[:, 0, :], ktile[0:64, :],
                                     qm[0:64, :], start=True, stop=True,
                                     tile_position=(0, 0))
                    nc.tensor.matmul(s2[:, 1, :], ktile[64:128, :],
                                     qm[64:128, :], start=True, stop=True,
                                     tile_position=(64, 0))
                    e2 = epool.tile([128, 2, QG], F16, name=f"e2_{i}",
                                    tag="e2")
                    if i >= len(iters) - 2 or i < 2:
                        # pipeline fill and drain iterations: split exp
                        # across both engines — shortens the critical chain
                        nc.scalar.activation(
                            e2[:, 0, :], s2[:, 0, :],
                            mybir.ActivationFunctionType.Exp, scale=0.125)
                        nc.vector.tensor_scalar(
                            e2[:, 1, :].bitcast(I16), s2[:, 1, :], A_DVE,
                            B_DVE, mybir.AluOpType.mult, mybir.AluOpType.add)
                    elif pick_engine() == 0:
                        nc.scalar.activation(
                            e2[:], s2[:],
                            mybir.ActivationFunctionType.Exp, scale=0.125)
                    else:
                        nc.vector.tensor_scalar(
                            e2[:].bitcast(I16), s2[:], A_DVE, B_DVE,
                            mybir.AluOpType.mult, mybir.AluOpType.add)
                    e2s[i] = e2
                if i >= LAG:
                    j = i - LAG
                    sgi, p, h, first, last = iters[j]
                    e2 = e2s.pop(j)
                    if first:
                        u_half[h] = upool.tile([128, QG], F32,
                                               name=f"u_{j}", tag="u")
                    ut = u_half[h]
                    # alternate chunk order by half so consecutive AVs share
                    # a stationary (B,h0 -> B,h1) when adjacent
                    chunks = ((0, 1) if h == 0 else (1, 0))
                    for ci, cc in enumerate(chunks):
                        nc.tensor.matmul(ut[:], vap(2 * p + cc),
                                         e2[:, cc, :],
                                         start=(first and ci == 0),
                                         stop=(last and ci == 1))
                    if last:
                        # evacuate this half as soon as its last AV is done
                        if h == 0:
                            u_out = usbp.tile([VC, 2, QG], F16,
                                              name=f"uo_{sgi}", tag="uo")
                            u_outs[sgi] = u_out
                        uo = u_outs[sgi]
                        if sgi == NSEG - 1 and splitfin:
                            # final segment: split the copy across both
                            # engines; one sync DMA per half (the scalar
                            # HWDGE queue measured 2x slower to issue)
                            nc.scalar.copy(uo[:, h, 0:256], ut[0:VC, 0:256])
                            nc.vector.tensor_copy(uo[:, h, 256:512],
                                                  ut[0:VC, 256:512])
                            nc.sync.dma_start(po[sgi][:, h, :], uo[:, h, :])
                        elif sgi == NSEG - 1:
                            if pick_engine() == 0:
                                nc.scalar.copy(uo[:, h, :], ut[0:VC, :])
                            else:
                                nc.vector.tensor_copy(uo[:, h, :], ut[0:VC, :])
                            nc.sync.dma_start(po[sgi][:, h, :], uo[:, h, :])
                        else:
                            if pick_engine() == 0:
                                nc.scalar.copy(uo[:, h, :], ut[0:VC, :])
                            else:
                                nc.vector.tensor_copy(uo[:, h, :], ut[0:VC, :])
                            if h == 1:
                                nc.sync.dma_start(po[sgi][:], uo[:])

    _dedup_waits(nc)
    _split_waits(nc)
    return nc


def _plan(valid_lens):
    """Pack (b, qh) blocks (weight = ceil(nchunks/2) pairs) into NCORE
    cores x segment slots. Returns (pattern, assign) where assign[c] is a
    list of (slot, b, qh, pair_lo, npairs) with slot < len(pattern)."""
    blocks = []
    for b in range(B):
        nch = max(1, math.ceil(int(valid_lens[b]) / 128))
        npair = (nch + 1) // 2
        for qh in range(2):
            blocks.append([npair, b, qh])
    total = sum(bl[0] for bl in blocks)
    S = max(1, math.ceil(total / NCORE))

    def try_pattern(pat):
        slots = []
        for c in range(NCORE):
            for si, cap in enumerate(pat):
                slots.append([cap, c, si])
        rem = sorted(([bl[0], bl[1], bl[2], 0] for bl in blocks),
                     reverse=True)
        assign = [[] for _ in range(NCORE)]
        waste = 0
        slots.sort(reverse=True)
        used = [False] * len(slots)
        for _ in range(1000):
            rem = [r for r in rem if r[0] > 0]
            if not rem:
                break
            rem.sort(reverse=True)
            r = rem[0]
            best = None
            for k, sl in enumerate(slots):
                if used[k]:
                    continue
                if best is None or sl[0] > slots[best][0]:
                    best = k
            if best is None:
                return None, None
            cap, c, si = slots[best]
            used[best] = True
            take = min(cap, r[0])
            assign[c].append((si, r[1], r[2], r[3], take))
            waste += cap - take
            r[3] += take
            r[0] -= take
        else:
            return None, None
        return waste, assign

    best = None
    cands = []
    for a in range(S, 0, -1):
        for bb in range(S - a, -1, -1):
            c = S - a - bb
            if bb and c > bb:
                continue
            if not bb and c:
                continue
            pat = tuple(x for x in (a, bb, c) if x > 0)
            cands.append(pat)
    for pat in cands:
        waste, assign = try_pattern(pat)
        if waste is None:
            continue
        key = (waste, len(pat))
        if best is None or key < best[0]:
            best = (key, pat, assign)
    assert best is not None
    return best[1], best[2]


def _host_prep(queries, keys, values, valid_lens, pattern, assign):
    queries = np.asarray(queries, dtype=np.float32)
    keys = np.asarray(keys, dtype=np.float32)
    values = np.asarray(values, dtype=np.float32)
    NSEG = len(pattern)
    S = sum(pattern)
    slot_lo = np.cumsum([0] + list(pattern))

    qts = queries.transpose(0, 2, 1).astype(np.float16)   # [B, 64, 2048]
    kts = keys.transpose(0, 2, 1).astype(np.float16)      # [B, 64, 2048]
    vps = np.zeros((B, NK + 256, VC), dtype=np.float16)   # [keys, 65]
    for b in range(B):
        L = int(valid_lens[b])
        vps[b, :L, :D] = values[b][:L].astype(np.float16)
        vps[b, :L, D] = 1.0

    offs, units, X = _layout(tuple(pattern))
    in_maps = []
    for c in range(NCORE):
        qblob = np.zeros((128, NSEG, QH), dtype=np.float16)
        kblob = np.zeros((128, S, 128), dtype=np.float16)
        vblob = np.zeros((128, 2 * S, VC), dtype=np.float16)
        for (si, b, qh, plo, np_) in assign[c]:
            qsl = qts[b][:, qh * QH : (qh + 1) * QH]
            qblob[0:64, si, :] = qsl
            # rows 64:128 are only DMA'd for seg 0 (dup_dma mode); filling
            # them always keeps the blob valid for either build flavor
            qblob[64:128, si, :] = qsl
            for pp in range(np_):
                gp = slot_lo[si] + pp           # global pair slot on core
                for half in range(2):
                    ch = (plo + pp) * 2 + half
                    c0, c1 = ch * 128, (ch + 1) * 128
                    c1v = min(c1, int(valid_lens[b]))
                    if c0 < NK:
                        kblob[64 * half : 64 * half + 64, gp, :] = (
                            kts[b][:, c0:c1])
                        # zero masked key columns (>= L)
                        if c1v < c1:
                            kblob[64 * half : 64 * half + 64, gp,
                                  max(0, c1v - c0):] = 0
                        vblob[:, 2 * gp + half, :] = vps[b][c0:c1, :]
        bxb = np.zeros((128, X), dtype=np.float16)
        for (kind, idx), off in offs.items():
            if kind == "q":
                bxb[:, off : off + QH] = qblob[:, idx, :]
            elif kind == "k":
                bxb[:, off : off + 128] = kblob[:, idx, :]
            else:
                bxb[:, off : off + VC] = vblob[:, idx, :]
        in_maps.append({"bx": bxb})
    return in_maps


def kernel(queries, keys, values, valid_lens):
    valid_lens = np.asarray(valid_lens)
    pattern, assign = _plan(valid_lens)
    key = tuple(pattern)
    if key not in _BUILT:
        _BUILT[key] = _build(pattern)
    in_maps = _host_prep(queries, keys, values, valid_lens, pattern, assign)
    res = run_bass_kernel_spmd(
        _BUILT[key],
        in_maps,
        core_ids=list(range(NCORE)),
        trace=bool(os.environ.get("KERNEL_TRACE")),
    )
    kernel.last_result = res

    # combine partials: ACC[b][qh] = sum over segments
    ACC = np.zeros((B, 2, VC, QH), dtype=np.float64)
    for c in range(NCORE):
        pc = np.asarray(res.results[c]["po"]).astype(np.float64)
        for (si, b, qh, plo, np_) in assign[c]:
            ACC[b, qh] += pc[si].reshape(VC, QH)
    out = np.empty((B, NQ, D), dtype=np.float32)
    for b in range(B):
        for qh in range(2):
            U = ACC[b, qh]
            out[b, qh * QH : (qh + 1) * QH, :] = (
                U[:D, :] / U[D : D + 1, :]).T.astype(np.float32)
    return out


kernel.last_result = None
